# revision 6
# baseline (speedup 1.0000x reference)
"""Trainium2 Bass kernel for ColumnConsistencyLoss (segment_reduce).

Problem: B=16, T=8192, C=128.
  probs = softmax(logits, -1)           # (N, C), N = B*T = 131072
  per column-id c (segment): n_c = #valid tokens, S_c = sum w*p, Q_c = sum w*p^2
  col_var_c = (sum_j Q_cj - sum_j S_cj^2 / n_safe_c) / (n_safe_c * C)
  loss = mean over columns with n_c > 1 of col_var_c

Sharding: data-parallel over tokens - each of the 8 cores processes
N/8 = 16384 tokens and produces partial per-segment accumulators
S (C x C) and Q (C x C).  Cross-core reduction of the tiny accumulators
plus final scalar math happens on the host (n_c via host bincount).

Device kernel per core (v5 - token-major, fp8 inputs, 2x everywhere):
  Host pre-quantizes logits to fp8e4m3 (~0.5% loss error vs 2e-2 budget),
  halving DMA bytes vs f32.  Token-major layout [p, j, c] keeps the
  matmul moving operand contiguous.  The softmax-normalize broadcast
  multiply - 1x in the naive form because the per-token scalar has
  free-step 0 - runs in DVE 2x mode via a pair-replication trick:
  r is materialized as [r|r] pairs (rr2), and the TT reads it with AP
  [p, j, (64 x step0), (2 x step1)] so the packed 16-bit reads stay
  step-1 / 4B-aligned.
    ScalarE: E = exp(L8) -> bf16                     [p, j, c]
    DVE:     d = sum_c E  (6 halving adds, bf16 2x) -> fp32
             r ~= 1/d (reciprocal_approx_fast); rr2 = [r|r] bf16
             rhs[:,:,0,:] = E * rr2-trick             (TT 2x)
             rhs[:,:,1,:] = rhs0^2                    (TT 2x; some chunks
                                                      on ScalarE Square)
    PE:      psum[(c),(s,c')] += M8_j^T @ rhs[:, j, :, :]  (F=256, fp32)
  The matmul contracts the 128 partitions (tokens); w rides in the fp8
  one-hot M8: psum[c,0,:] = S_c, psum[c,1,:] = Q_c.
"""

import numpy as np
import ml_dtypes

NCORES = 8
P = 128           # partitions
C = 128           # columns / segments
H = C // 2        # 64
B, T = 16, 8192
N_TOK = B * T
TOK_PER_CORE = N_TOK // NCORES   # 16384
J_FULL = TOK_PER_CORE // P       # 128 token tiles per core
CHUNKS = (12, 24, 28, 28, 24, 12)      # token tiles per DMA/compute chunk
SQ_SCALAR = (False, True, False, True, False, True)  # Square on ScalarE?

TRACE = False          # set True (e.g. from test.py) to capture NTFF profile
TRACE_TMPDIR = None    # where trace/NEFF artifacts land when TRACE is set
LAST_RESULT = None     # BassKernelResults of the last run (for profiling)

_NC_CACHE = {}


def build_nc(chunks=CHUNKS, sq_scalar=SQ_SCALAR):
    """Build + compile the Bass program (SPMD; same NEFF on all cores)."""
    from concourse import bacc, mybir
    import concourse.tile as tile

    f32 = mybir.dt.float32
    bf16 = mybir.dt.bfloat16
    fp8 = mybir.dt.float8e4
    Exp = mybir.ActivationFunctionType.Exp
    Square = mybir.ActivationFunctionType.Square
    Alu = mybir.AluOpType

    j_full = sum(chunks)
    tok = j_full * P

    nc = bacc.Bacc("TRN2", target_bir_lowering=False, debug=False,
                   enable_asserts=False)

    lg_d = nc.dram_tensor("lg8", [tok, C], fp8, kind="ExternalInput")
    m8_d = nc.dram_tensor("m8", [tok, C], fp8, kind="ExternalInput")
    sq_d = nc.dram_tensor("sq_out", [C, 2, C], f32, kind="ExternalOutput")

    with tile.TileContext(nc) as tc:
        with (
            tc.tile_pool(name="const", bufs=1) as constp,
            tc.tile_pool(name="ld", bufs=3) as ldp,
            tc.tile_pool(name="ep", bufs=3) as ep,
            tc.tile_pool(name="rp", bufs=3) as rp,
            tc.tile_pool(name="tp", bufs=2) as tp,
            tc.tile_pool(name="sp", bufs=3) as sp,
            tc.tile_pool(name="psum", bufs=1, space="PSUM") as psump,
        ):
            psum_sq = psump.tile([C, 2, C], f32)

            # DRAM views: (p, j, c) with token t = p*j_full + j
            lg_ap = lg_d[:].rearrange("(p j) c -> p j c", j=j_full)
            m8_ap = m8_d[:].rearrange("(p j) c -> p j c", j=j_full)

            nchunk = len(chunks)
            offs = [sum(chunks[:k]) for k in range(nchunk)]
            Ls = [None] * nchunk
            Ms = [None] * nchunk
            Es = [None] * nchunk

            def emit_load(k):
                cj = chunks[k]
                o = offs[k]
                L = ldp.tile([P, cj, C], fp8, tag="L")
                nc.sync.dma_start(L[:], lg_ap[:, o:o + cj, :])
                M8 = ldp.tile([P, cj, C], fp8, tag="M8")
                # gpsimd-issued DMA keeps the one-hot stream off the busy
                # Scalar queue and off the logits HWDGE ring
                nc.gpsimd.dma_start(M8[:], m8_ap[:, o:o + cj, :])
                Ls[k], Ms[k] = L, M8

            def emit_exp(k):
                cj = chunks[k]
                E = ep.tile([P, cj, C], bf16, tag="E")
                nc.scalar.activation(E[:], Ls[k][:], Exp)
                Es[k] = E

            emit_load(0)
            emit_load(1)
            emit_exp(0)
            for k, cj in enumerate(chunks):
                if k + 2 < nchunk:
                    emit_load(k + 2)
                E, M8, o = Es[k], Ms[k], offs[k]
                # d = rowsum over c: halving-add tree, bf16 2x mode.
                # All levels live in ONE scratch tile (offsets 0,64,96,...)
                # so Tile emits a single release instead of six.
                ht = tp.tile([P, cj, 2 * H], bf16, tag="ht")
                nc.vector.tensor_tensor(
                    ht[:, :, 0:H], E[:, :, 0:H], E[:, :, H:C], op=Alu.add)
                a, w = 0, H
                while w > 2:
                    w //= 2
                    nc.vector.tensor_tensor(
                        ht[:, :, a + 2 * w:a + 3 * w],
                        ht[:, :, a:a + w], ht[:, :, a + w:a + 2 * w],
                        op=Alu.add)
                    a += 2 * w
                dr = sp.tile([P, 2, cj], f32, tag="dr")
                nc.vector.tensor_tensor(dr[:, 0, :], ht[:, :, a],
                                        ht[:, :, a + 1], op=Alu.add)
                r = dr[:, 1, :]
                nc.vector.reciprocal_approx_fast(r, dr[:, 0, :])
                # [r|r] pairs so the normalize TT can read r with a
                # step-1 inner AP (keeps DVE 2x mode)
                rr2 = sp.tile([P, cj, 2], bf16, tag="rr2")
                nc.vector.tensor_copy(
                    rr2[:], r[:, :, None].to_broadcast([P, cj, 2]))

                if k + 1 < nchunk:
                    emit_exp(k + 1)

                rhs = rp.tile([P, cj, 2, C], bf16, tag="rhs")
                nc.vector.tensor_tensor(
                    rhs[:, :, 0, :].rearrange("p j (a b) -> p j a b", b=2),
                    E[:].rearrange("p j (a b) -> p j a b", b=2),
                    rr2[:, :, None, :].to_broadcast([P, cj, H, 2]),
                    op=Alu.mult)
                if sq_scalar[k]:
                    nc.scalar.activation(rhs[:, :, 1, :], rhs[:, :, 0, :],
                                         Square)
                else:
                    nc.vector.tensor_tensor(
                        rhs[:, :, 1, :], rhs[:, :, 0, :], rhs[:, :, 0, :],
                        op=Alu.mult)
                for jj in range(cj):
                    j = o + jj
                    nc.tensor.matmul(
                        psum_sq[:], M8[:, jj, :], rhs[:, jj, :, :],
                        start=(j == 0), stop=(j == j_full - 1))

            out_t = constp.tile([C, 2, C], f32)
            nc.vector.tensor_copy(out_t[:], psum_sq[:])
            nc.sync.dma_start(sq_d[:], out_t[:])

    nc.compile()
    return nc


def _get_nc():
    key = (CHUNKS, SQ_SCALAR)
    if key not in _NC_CACHE:
        _NC_CACHE[key] = build_nc(CHUNKS, SQ_SCALAR)
    return _NC_CACHE[key]


def kernel(column_logits, column_assignments, valid_mask):
    global LAST_RESULT
    from concourse.bass_utils import run_bass_kernel_spmd

    logits = np.asarray(column_logits, dtype=np.float32).reshape(N_TOK, C)
    seg = np.asarray(column_assignments).reshape(N_TOK).astype(np.int64)
    w = np.asarray(valid_mask).reshape(N_TOK).astype(bool)

    fp8np = ml_dtypes.float8_e4m3
    lg8 = logits.astype(fp8np)
    M8_full = np.zeros((N_TOK, C), dtype=fp8np)
    M8_full[np.arange(N_TOK)[w], seg[w]] = fp8np(1.0)   # w folded into M

    in_maps = []
    for i in range(NCORES):
        sl = slice(i * TOK_PER_CORE, (i + 1) * TOK_PER_CORE)
        in_maps.append({
            "lg8": np.ascontiguousarray(lg8[sl]),
            "m8": np.ascontiguousarray(M8_full[sl]),
        })

    nc = _get_nc()
    res = run_bass_kernel_spmd(nc, in_maps, list(range(NCORES)), trace=TRACE,
                               tmpdir=TRACE_TMPDIR)
    LAST_RESULT = res

    SQ = np.zeros((C, 2, C), np.float64)
    for rm in res.results:
        SQ += np.asarray(rm["sq_out"], dtype=np.float64)
    S = SQ[:, 0, :]
    Q = SQ[:, 1, :]

    n = np.bincount(seg[w], minlength=C).astype(np.float64)
    n_safe = np.maximum(n, 1.0)
    ssd_sum = Q.sum(axis=1) - (S * S).sum(axis=1) / n_safe
    col_var = ssd_sum / (n_safe * C)
    has_multi = n > 1.0
    count = has_multi.sum()
    total = np.where(has_multi, col_var, 0.0).sum()
    loss = total / max(count, 1.0) if count > 0 else 0.0
    return np.asarray(loss, dtype=np.float32)


# revision 9
# speedup vs baseline: 1.0139x; 1.0139x over previous
"""Trainium2 Bass kernel for ColumnConsistencyLoss (segment_reduce).

Problem: B=16, T=8192, C=128.
  probs = softmax(logits, -1)           # (N, C), N = B*T = 131072
  per column-id c (segment): n_c = #valid tokens, S_c = sum w*p, Q_c = sum w*p^2
  col_var_c = (sum_j Q_cj - sum_j S_cj^2 / n_safe_c) / (n_safe_c * C)
  loss = mean over columns with n_c > 1 of col_var_c

Sharding: data-parallel over tokens - each of the 8 cores processes
N/8 = 16384 tokens and produces partial per-segment accumulators
S (C x C) and Q (C x C).  Cross-core reduction of the tiny accumulators
plus final scalar math happens on the host (n_c via host bincount).

Device kernel per core (v5 - token-major, fp8 inputs, 2x everywhere):
  Host pre-quantizes logits to fp8e4m3 (~0.5% loss error vs 2e-2 budget),
  halving DMA bytes vs f32.  Token-major layout [p, j, c] keeps the
  matmul moving operand contiguous.  The softmax-normalize broadcast
  multiply - 1x in the naive form because the per-token scalar has
  free-step 0 - runs in DVE 2x mode via a pair-replication trick:
  r is materialized as [r|r] pairs (rr2), and the TT reads it with AP
  [p, j, (64 x step0), (2 x step1)] so the packed 16-bit reads stay
  step-1 / 4B-aligned.
    ScalarE: E = exp(L8) -> bf16                     [p, j, c]
    DVE:     d = sum_c E  (6 halving adds, bf16 2x) -> fp32
             r ~= 1/d (reciprocal_approx_fast); rr2 = [r|r] bf16
             rhs[:,:,0,:] = E * rr2-trick             (TT 2x)
             rhs[:,:,1,:] = rhs0^2                    (TT 2x; some chunks
                                                      on ScalarE Square)
    PE:      psum[(c),(s,c')] += M8_j^T @ rhs[:, j, :, :]  (F=256, fp32)
  The matmul contracts the 128 partitions (tokens); w rides in the fp8
  one-hot M8: psum[c,0,:] = S_c, psum[c,1,:] = Q_c.
"""

import numpy as np
import ml_dtypes

NCORES = 8
P = 128           # partitions
C = 128           # columns / segments
H = C // 2        # 64
B, T = 16, 8192
N_TOK = B * T
TOK_PER_CORE = N_TOK // NCORES   # 16384
J_FULL = TOK_PER_CORE // P       # 128 token tiles per core
CHUNKS = (12, 24, 28, 28, 24, 12)      # token tiles per DMA/compute chunk
SQ_SCALAR = (False, True, False, True, False, True)  # Square on ScalarE?

TRACE = False          # set True (e.g. from test.py) to capture NTFF profile
TRACE_TMPDIR = None    # where trace/NEFF artifacts land when TRACE is set
LAST_RESULT = None     # BassKernelResults of the last run (for profiling)

_NC_CACHE = {}


def build_nc(chunks=CHUNKS, sq_scalar=SQ_SCALAR):
    """Build + compile the Bass program (SPMD; same NEFF on all cores)."""
    from concourse import bacc, mybir
    import concourse.tile as tile

    f32 = mybir.dt.float32
    bf16 = mybir.dt.bfloat16
    fp8 = mybir.dt.float8e4
    Exp = mybir.ActivationFunctionType.Exp
    Square = mybir.ActivationFunctionType.Square
    Alu = mybir.AluOpType

    j_full = sum(chunks)
    tok = j_full * P

    nc = bacc.Bacc("TRN2", target_bir_lowering=False, debug=False,
                   enable_asserts=False)

    lg_d = nc.dram_tensor("lg8", [tok, C], fp8, kind="ExternalInput")
    m8_d = nc.dram_tensor("m8", [tok, C], fp8, kind="ExternalInput")
    sq_d = nc.dram_tensor("sq_out", [C, 2, C], f32, kind="ExternalOutput")

    with tile.TileContext(nc) as tc:
        with (
            tc.tile_pool(name="const", bufs=1) as constp,
            tc.tile_pool(name="ld", bufs=6) as ldp,
            tc.tile_pool(name="ep", bufs=3) as ep,
            tc.tile_pool(name="rp", bufs=3) as rp,
            tc.tile_pool(name="tp", bufs=2) as tp,
            tc.tile_pool(name="sp", bufs=3) as sp,
            tc.tile_pool(name="psum", bufs=1, space="PSUM") as psump,
        ):
            psum_sq = psump.tile([C, 2, C], f32)

            # DRAM views: (p, j, c) with token t = p*j_full + j
            lg_ap = lg_d[:].rearrange("(p j) c -> p j c", j=j_full)
            m8_ap = m8_d[:].rearrange("(p j) c -> p j c", j=j_full)

            nchunk = len(chunks)
            offs = [sum(chunks[:k]) for k in range(nchunk)]
            Ls = [None] * nchunk
            Ms = [None] * nchunk
            Es = [None] * nchunk

            def emit_load(k):
                cj = chunks[k]
                o = offs[k]
                L = ldp.tile([P, cj, C], fp8, tag="L")
                nc.sync.dma_start(L[:], lg_ap[:, o:o + cj, :])
                M8 = ldp.tile([P, cj, C], fp8, tag="M8")
                # gpsimd-issued DMA keeps the one-hot stream off the busy
                # Scalar queue and off the logits HWDGE ring
                nc.gpsimd.dma_start(M8[:], m8_ap[:, o:o + cj, :])
                Ls[k], Ms[k] = L, M8

            def emit_exp(k):
                cj = chunks[k]
                E = ep.tile([P, cj, C], bf16, tag="E")
                nc.scalar.activation(E[:], Ls[k][:], Exp)
                Es[k] = E

            for k in range(nchunk):
                emit_load(k)
            emit_exp(0)
            for k, cj in enumerate(chunks):
                E, M8, o = Es[k], Ms[k], offs[k]
                # d = rowsum over c: halving-add tree, bf16 2x mode.
                # All levels live in ONE scratch tile (offsets 0,64,96,...)
                # so Tile emits a single release instead of six.
                ht = tp.tile([P, cj, 2 * H], bf16, tag="ht")
                nc.vector.tensor_tensor(
                    ht[:, :, 0:H], E[:, :, 0:H], E[:, :, H:C], op=Alu.add)
                a, w = 0, H
                while w > 2:
                    w //= 2
                    nc.vector.tensor_tensor(
                        ht[:, :, a + 2 * w:a + 3 * w],
                        ht[:, :, a:a + w], ht[:, :, a + w:a + 2 * w],
                        op=Alu.add)
                    a += 2 * w
                dr = sp.tile([P, 2, cj], f32, tag="dr")
                nc.vector.tensor_tensor(dr[:, 0, :], ht[:, :, a],
                                        ht[:, :, a + 1], op=Alu.add)
                r = dr[:, 1, :]
                nc.vector.reciprocal_approx_fast(r, dr[:, 0, :])
                # [r|r] pairs so the normalize TT can read r with a
                # step-1 inner AP (keeps DVE 2x mode)
                rr2 = sp.tile([P, cj, 2], bf16, tag="rr2")
                nc.vector.tensor_copy(
                    rr2[:], r[:, :, None].to_broadcast([P, cj, 2]))

                if k + 1 < nchunk:
                    emit_exp(k + 1)

                rhs = rp.tile([P, cj, 2, C], bf16, tag="rhs")
                nc.vector.tensor_tensor(
                    rhs[:, :, 0, :].rearrange("p j (a b) -> p j a b", b=2),
                    E[:].rearrange("p j (a b) -> p j a b", b=2),
                    rr2[:, :, None, :].to_broadcast([P, cj, H, 2]),
                    op=Alu.mult)
                if sq_scalar[k]:
                    nc.scalar.activation(rhs[:, :, 1, :], rhs[:, :, 0, :],
                                         Square)
                else:
                    nc.vector.tensor_tensor(
                        rhs[:, :, 1, :], rhs[:, :, 0, :], rhs[:, :, 0, :],
                        op=Alu.mult)
                for jj in range(cj):
                    j = o + jj
                    nc.tensor.matmul(
                        psum_sq[:], M8[:, jj, :], rhs[:, jj, :, :],
                        start=(j == 0), stop=(j == j_full - 1))

            out_t = constp.tile([C, 2, C], f32)
            nc.vector.tensor_copy(out_t[:], psum_sq[:])
            nc.sync.dma_start(sq_d[:], out_t[:])

    nc.compile()
    return nc


def _get_nc():
    key = (CHUNKS, SQ_SCALAR)
    if key not in _NC_CACHE:
        _NC_CACHE[key] = build_nc(CHUNKS, SQ_SCALAR)
    return _NC_CACHE[key]


def kernel(column_logits, column_assignments, valid_mask):
    global LAST_RESULT
    from concourse.bass_utils import run_bass_kernel_spmd

    logits = np.asarray(column_logits, dtype=np.float32).reshape(N_TOK, C)
    seg = np.asarray(column_assignments).reshape(N_TOK).astype(np.int64)
    w = np.asarray(valid_mask).reshape(N_TOK).astype(bool)

    fp8np = ml_dtypes.float8_e4m3
    lg8 = logits.astype(fp8np)
    M8_full = np.zeros((N_TOK, C), dtype=fp8np)
    M8_full[np.arange(N_TOK)[w], seg[w]] = fp8np(1.0)   # w folded into M

    in_maps = []
    for i in range(NCORES):
        sl = slice(i * TOK_PER_CORE, (i + 1) * TOK_PER_CORE)
        in_maps.append({
            "lg8": np.ascontiguousarray(lg8[sl]),
            "m8": np.ascontiguousarray(M8_full[sl]),
        })

    nc = _get_nc()
    res = run_bass_kernel_spmd(nc, in_maps, list(range(NCORES)), trace=TRACE,
                               tmpdir=TRACE_TMPDIR)
    LAST_RESULT = res

    SQ = np.zeros((C, 2, C), np.float64)
    for rm in res.results:
        SQ += np.asarray(rm["sq_out"], dtype=np.float64)
    S = SQ[:, 0, :]
    Q = SQ[:, 1, :]

    n = np.bincount(seg[w], minlength=C).astype(np.float64)
    n_safe = np.maximum(n, 1.0)
    ssd_sum = Q.sum(axis=1) - (S * S).sum(axis=1) / n_safe
    col_var = ssd_sum / (n_safe * C)
    has_multi = n > 1.0
    count = has_multi.sum()
    total = np.where(has_multi, col_var, 0.0).sum()
    loss = total / max(count, 1.0) if count > 0 else 0.0
    return np.asarray(loss, dtype=np.float32)


# revision 14
# speedup vs baseline: 1.0163x; 1.0024x over previous
"""Trainium2 Bass kernel for ColumnConsistencyLoss (segment_reduce).

Problem: B=16, T=8192, C=128.
  probs = softmax(logits, -1)           # (N, C), N = B*T = 131072
  per column-id c (segment): n_c = #valid tokens, S_c = sum w*p, Q_c = sum w*p^2
  col_var_c = (sum_j Q_cj - sum_j S_cj^2 / n_safe_c) / (n_safe_c * C)
  loss = mean over columns with n_c > 1 of col_var_c

Sharding: data-parallel over tokens - each of the 8 cores processes
N/8 = 16384 tokens and produces partial per-segment accumulators
S (C x C) and Q (C x C).  Cross-core reduction of the tiny accumulators
plus final scalar math happens on the host (n_c via host bincount).

Device kernel per core (v5 - token-major, fp8 inputs, 2x everywhere):
  Host pre-quantizes logits to fp8e4m3 (~0.5% loss error vs 2e-2 budget),
  halving DMA bytes vs f32.  Token-major layout [p, j, c] keeps the
  matmul moving operand contiguous.  The softmax-normalize broadcast
  multiply - 1x in the naive form because the per-token scalar has
  free-step 0 - runs in DVE 2x mode via a pair-replication trick:
  r is materialized as [r|r] pairs (rr2), and the TT reads it with AP
  [p, j, (64 x step0), (2 x step1)] so the packed 16-bit reads stay
  step-1 / 4B-aligned.
    ScalarE: E = exp(L8) -> bf16                     [p, j, c]
    DVE:     d = sum_c E  (6 halving adds, bf16 2x) -> fp32
             r ~= 1/d (reciprocal_approx_fast); rr2 = [r|r] bf16
             rhs[:,:,0,:] = E * rr2-trick             (TT 2x)
             rhs[:,:,1,:] = rhs0^2                    (TT 2x; some chunks
                                                      on ScalarE Square)
    PE:      psum[(c),(s,c')] += M8_j^T @ rhs[:, j, :, :]  (F=256, fp32)
  The matmul contracts the 128 partitions (tokens); w rides in the fp8
  one-hot M8: psum[c,0,:] = S_c, psum[c,1,:] = Q_c.
"""

import numpy as np
import ml_dtypes

NCORES = 8
P = 128           # partitions
C = 128           # columns / segments
H = C // 2        # 64
B, T = 16, 8192
N_TOK = B * T
TOK_PER_CORE = N_TOK // NCORES   # 16384
J_FULL = TOK_PER_CORE // P       # 128 token tiles per core
CHUNKS = (12, 24, 28, 28, 24, 12)      # token tiles per DMA/compute chunk
SQ_SCALAR = (False, True, False, True, False, False)  # Square on ScalarE?

TRACE = False          # set True (e.g. from test.py) to capture NTFF profile
TRACE_TMPDIR = None    # where trace/NEFF artifacts land when TRACE is set
LAST_RESULT = None     # BassKernelResults of the last run (for profiling)

_NC_CACHE = {}


def build_nc(chunks=CHUNKS, sq_scalar=SQ_SCALAR):
    """Build + compile the Bass program (SPMD; same NEFF on all cores)."""
    from concourse import bacc, mybir
    import concourse.tile as tile

    f32 = mybir.dt.float32
    bf16 = mybir.dt.bfloat16
    fp8 = mybir.dt.float8e4
    Exp = mybir.ActivationFunctionType.Exp
    Square = mybir.ActivationFunctionType.Square
    Alu = mybir.AluOpType

    j_full = sum(chunks)
    tok = j_full * P

    nc = bacc.Bacc("TRN2", target_bir_lowering=False, debug=False,
                   enable_asserts=False)

    # flat [P, K] layout, chunk-blocked: each (partition, chunk) slab is
    # cj*C contiguous bytes so SDMA runs at line rate (>=512B descriptors;
    # per-token 128B fp8 rows would throttle to ~51 GB/s)
    lg_d = nc.dram_tensor("lg8", [P, j_full * C], fp8, kind="ExternalInput")
    m8_d = nc.dram_tensor("m8", [P, j_full * C], fp8, kind="ExternalInput")
    sq_d = nc.dram_tensor("sq_out", [C, 2, C], f32, kind="ExternalOutput")

    with tile.TileContext(nc) as tc:
        with (
            tc.tile_pool(name="const", bufs=1) as constp,
            tc.tile_pool(name="ld", bufs=6) as ldp,
            tc.tile_pool(name="ep", bufs=3) as ep,
            tc.tile_pool(name="rp", bufs=3) as rp,
            tc.tile_pool(name="tp", bufs=2) as tp,
            tc.tile_pool(name="sp", bufs=3) as sp,
            tc.tile_pool(name="psum", bufs=1, space="PSUM") as psump,
        ):
            psum_sq = psump.tile([C, 2, C], f32)

            nchunk = len(chunks)
            offs = [sum(chunks[:k]) for k in range(nchunk)]
            Ls = [None] * nchunk
            Ms = [None] * nchunk
            Es = [None] * nchunk

            def emit_load(k):
                cj = chunks[k]
                o = offs[k]
                L = ldp.tile([P, cj, C], fp8, tag="L")
                nc.sync.dma_start(
                    L[:], lg_d[:, o * C:(o + cj) * C]
                    .rearrange("p (j c) -> p j c", c=C))
                M8 = ldp.tile([P, cj, C], fp8, tag="M8")
                # gpsimd-issued DMA keeps the one-hot stream off the busy
                # Scalar queue and off the logits HWDGE ring
                nc.gpsimd.dma_start(
                    M8[:], m8_d[:, o * C:(o + cj) * C]
                    .rearrange("p (j c) -> p j c", c=C))
                Ls[k], Ms[k] = L, M8

            def emit_exp(k):
                cj = chunks[k]
                E = ep.tile([P, cj, C], bf16, tag="E")
                nc.scalar.activation(E[:], Ls[k][:], Exp)
                Es[k] = E

            for k in range(nchunk):
                emit_load(k)
            emit_exp(0)
            for k, cj in enumerate(chunks):
                E, M8, o = Es[k], Ms[k], offs[k]
                # d = rowsum over c: halving-add tree, bf16 2x mode.
                # All levels live in ONE scratch tile (offsets 0,64,96,...)
                # so Tile emits a single release instead of six.
                ht = tp.tile([P, cj, 2 * H], bf16, tag="ht")
                nc.vector.tensor_tensor(
                    ht[:, :, 0:H], E[:, :, 0:H], E[:, :, H:C], op=Alu.add)
                a, w = 0, H
                while w > 2:
                    w //= 2
                    nc.vector.tensor_tensor(
                        ht[:, :, a + 2 * w:a + 3 * w],
                        ht[:, :, a:a + w], ht[:, :, a + w:a + 2 * w],
                        op=Alu.add)
                    a += 2 * w
                dr = sp.tile([P, 2, cj], f32, tag="dr")
                nc.vector.tensor_tensor(dr[:, 0, :], ht[:, :, a],
                                        ht[:, :, a + 1], op=Alu.add)
                r = dr[:, 1, :]
                nc.vector.reciprocal_approx_fast(r, dr[:, 0, :])
                # [r|r] pairs so the normalize TT can read r with a
                # step-1 inner AP (keeps DVE 2x mode)
                rr2 = sp.tile([P, cj, 2], bf16, tag="rr2")
                nc.vector.tensor_copy(
                    rr2[:], r[:, :, None].to_broadcast([P, cj, 2]))

                if k + 1 < nchunk:
                    emit_exp(k + 1)

                rhs = rp.tile([P, cj, 2, C], bf16, tag="rhs")
                nc.vector.tensor_tensor(
                    rhs[:, :, 0, :].rearrange("p j (a b) -> p j a b", b=2),
                    E[:].rearrange("p j (a b) -> p j a b", b=2),
                    rr2[:, :, None, :].to_broadcast([P, cj, H, 2]),
                    op=Alu.mult)
                if sq_scalar[k]:
                    nc.scalar.activation(rhs[:, :, 1, :], rhs[:, :, 0, :],
                                         Square)
                else:
                    nc.vector.tensor_tensor(
                        rhs[:, :, 1, :], rhs[:, :, 0, :], rhs[:, :, 0, :],
                        op=Alu.mult)
                for jj in range(cj):
                    j = o + jj
                    nc.tensor.matmul(
                        psum_sq[:], M8[:, jj, :], rhs[:, jj, :, :],
                        start=(j == 0), stop=(j == j_full - 1))

            out_t = constp.tile([C, 2, C], f32)
            nc.vector.tensor_copy(out_t[:], psum_sq[:])
            nc.sync.dma_start(sq_d[:], out_t[:])

    nc.compile()
    return nc


def _get_nc():
    key = (CHUNKS, SQ_SCALAR)
    if key not in _NC_CACHE:
        _NC_CACHE[key] = build_nc(CHUNKS, SQ_SCALAR)
    return _NC_CACHE[key]


def kernel(column_logits, column_assignments, valid_mask):
    global LAST_RESULT
    from concourse.bass_utils import run_bass_kernel_spmd

    logits = np.asarray(column_logits, dtype=np.float32).reshape(N_TOK, C)
    seg = np.asarray(column_assignments).reshape(N_TOK).astype(np.int64)
    w = np.asarray(valid_mask).reshape(N_TOK).astype(bool)

    fp8np = ml_dtypes.float8_e4m3
    lg8 = logits.astype(fp8np)
    M8_full = np.zeros((N_TOK, C), dtype=fp8np)
    M8_full[np.arange(N_TOK)[w], seg[w]] = fp8np(1.0)   # w folded into M

    offs = [sum(CHUNKS[:k]) for k in range(len(CHUNKS))]
    in_maps = []
    for i in range(NCORES):
        sl = slice(i * TOK_PER_CORE, (i + 1) * TOK_PER_CORE)
        lgc = lg8[sl].reshape(P, J_FULL, C)      # token t = p*J_FULL + j
        m8c = M8_full[sl].reshape(P, J_FULL, C)
        lg_flat = np.empty((P, J_FULL * C), dtype=fp8np)
        m8_flat = np.empty((P, J_FULL * C), dtype=fp8np)
        for o, cj in zip(offs, CHUNKS):
            lg_flat[:, o * C:(o + cj) * C] = lgc[:, o:o + cj, :].reshape(P, cj * C)
            m8_flat[:, o * C:(o + cj) * C] = m8c[:, o:o + cj, :].reshape(P, cj * C)
        in_maps.append({"lg8": lg_flat, "m8": m8_flat})

    nc = _get_nc()
    res = run_bass_kernel_spmd(nc, in_maps, list(range(NCORES)), trace=TRACE,
                               tmpdir=TRACE_TMPDIR)
    LAST_RESULT = res

    SQ = np.zeros((C, 2, C), np.float64)
    for rm in res.results:
        SQ += np.asarray(rm["sq_out"], dtype=np.float64)
    S = SQ[:, 0, :]
    Q = SQ[:, 1, :]

    n = np.bincount(seg[w], minlength=C).astype(np.float64)
    n_safe = np.maximum(n, 1.0)
    ssd_sum = Q.sum(axis=1) - (S * S).sum(axis=1) / n_safe
    col_var = ssd_sum / (n_safe * C)
    has_multi = n > 1.0
    count = has_multi.sum()
    total = np.where(has_multi, col_var, 0.0).sum()
    loss = total / max(count, 1.0) if count > 0 else 0.0
    return np.asarray(loss, dtype=np.float32)


# revision 18
# speedup vs baseline: 1.1017x; 1.0841x over previous
"""Trainium2 Bass kernel for ColumnConsistencyLoss (segment_reduce).

Problem: B=16, T=8192, C=128.
  probs = softmax(logits, -1)           # (N, C), N = B*T = 131072
  per column-id c (segment): n_c = #valid tokens, S_c = sum w*p, Q_c = sum w*p^2
  col_var_c = (sum_j Q_cj - sum_j S_cj^2 / n_safe_c) / (n_safe_c * C)
  loss = mean over columns with n_c > 1 of col_var_c

Sharding: data-parallel over tokens - each of the 8 cores processes
N/8 = 16384 tokens and produces partial per-segment accumulators
S (C x C) and Q (C x C).  Cross-core reduction of the tiny accumulators
plus final scalar math happens on the host (n_c via host bincount).

Device kernel per core (v5 - token-major, fp8 inputs, 2x everywhere):
  Host pre-quantizes logits to fp8e4m3 (~0.5% loss error vs 2e-2 budget),
  halving DMA bytes vs f32.  Token-major layout [p, j, c] keeps the
  matmul moving operand contiguous.  The softmax-normalize broadcast
  multiply - 1x in the naive form because the per-token scalar has
  free-step 0 - runs in DVE 2x mode via a pair-replication trick:
  r is materialized as [r|r] pairs (rr2), and the TT reads it with AP
  [p, j, (64 x step0), (2 x step1)] so the packed 16-bit reads stay
  step-1 / 4B-aligned.
    ScalarE: E = exp(L8) -> bf16                     [p, j, c]
    DVE:     d = sum_c E  (6 halving adds, bf16 2x) -> fp32
             r ~= 1/d (reciprocal_approx_fast); rr2 = [r|r] bf16
             rhs[:,:,0,:] = E * rr2-trick             (TT 2x)
             rhs[:,:,1,:] = rhs0^2                    (TT 2x; some chunks
                                                      on ScalarE Square)
    PE:      psum[(c),(s,c')] += M8_j^T @ rhs[:, j, :, :]  (F=256, fp32)
  The matmul contracts the 128 partitions (tokens); w rides in the fp8
  one-hot M8: psum[c,0,:] = S_c, psum[c,1,:] = Q_c.
"""

import numpy as np
import ml_dtypes

NCORES = 8
P = 128           # partitions
C = 128           # columns / segments
H = C // 2        # 64
B, T = 16, 8192
N_TOK = B * T
TOK_PER_CORE = N_TOK // NCORES   # 16384
J_FULL = TOK_PER_CORE // P       # 128 token tiles per core
CHUNKS = (12, 24, 28, 28, 24, 12)      # token tiles per DMA/compute chunk
SQ_FRAC = 0.60    # fraction of each chunk's squares done on ScalarE

TRACE = False          # set True (e.g. from test.py) to capture NTFF profile
TRACE_TMPDIR = None    # where trace/NEFF artifacts land when TRACE is set
LAST_RESULT = None     # BassKernelResults of the last run (for profiling)

_NC_CACHE = {}


def build_nc(chunks=CHUNKS, sq_frac=SQ_FRAC):
    """Build + compile the Bass program (SPMD; same NEFF on all cores)."""
    from concourse import bacc, mybir
    import concourse.tile as tile

    f32 = mybir.dt.float32
    bf16 = mybir.dt.bfloat16
    fp8 = mybir.dt.float8e4
    Exp = mybir.ActivationFunctionType.Exp
    Square = mybir.ActivationFunctionType.Square
    Alu = mybir.AluOpType

    j_full = sum(chunks)
    tok = j_full * P

    nc = bacc.Bacc("TRN2", target_bir_lowering=False, debug=False,
                   enable_asserts=False)

    # flat [P, K] layout, chunk-blocked: each (partition, chunk) slab is
    # cj*C contiguous bytes so SDMA runs at line rate (>=512B descriptors;
    # per-token 128B fp8 rows would throttle to ~51 GB/s)
    lg_d = nc.dram_tensor("lg8", [P, j_full * C], fp8, kind="ExternalInput")
    m8_d = nc.dram_tensor("m8", [P, j_full * C], fp8, kind="ExternalInput")
    sq_d = nc.dram_tensor("sq_out", [C, 2, C], f32, kind="ExternalOutput")

    with tile.TileContext(nc) as tc:
        with (
            tc.tile_pool(name="const", bufs=1) as constp,
            tc.tile_pool(name="ld", bufs=6) as ldp,
            tc.tile_pool(name="ep", bufs=3) as ep,
            tc.tile_pool(name="rp", bufs=3) as rp,
            tc.tile_pool(name="tp", bufs=2) as tp,
            tc.tile_pool(name="sp", bufs=3) as sp,
            tc.tile_pool(name="psum", bufs=1, space="PSUM") as psump,
        ):
            psum_sq = psump.tile([C, 2, C], f32)

            nchunk = len(chunks)
            offs = [sum(chunks[:k]) for k in range(nchunk)]
            Ls = [None] * nchunk
            Ms = [None] * nchunk
            Es = [None] * nchunk

            def emit_load(k):
                cj = chunks[k]
                o = offs[k]
                L = ldp.tile([P, cj, C], fp8, tag="L")
                nc.sync.dma_start(
                    L[:], lg_d[:, o * C:(o + cj) * C]
                    .rearrange("p (j c) -> p j c", c=C))
                M8 = ldp.tile([P, cj, C], fp8, tag="M8")
                # gpsimd-issued DMA keeps the one-hot stream off the busy
                # Scalar queue and off the logits HWDGE ring
                nc.gpsimd.dma_start(
                    M8[:], m8_d[:, o * C:(o + cj) * C]
                    .rearrange("p (j c) -> p j c", c=C))
                Ls[k], Ms[k] = L, M8

            def emit_exp(k):
                cj = chunks[k]
                E = ep.tile([P, cj, C], bf16, tag="E")
                nc.scalar.activation(E[:], Ls[k][:], Exp)
                Es[k] = E

            for k in range(nchunk):
                emit_load(k)
            emit_exp(0)
            for k, cj in enumerate(chunks):
                E, M8, o = Es[k], Ms[k], offs[k]
                # d = rowsum over c: halving-add tree, bf16 2x mode.
                # All levels live in ONE scratch tile (offsets 0,64,96,...)
                # so Tile emits a single release instead of six.
                ht = tp.tile([P, cj, 2 * H], bf16, tag="ht")
                nc.vector.tensor_tensor(
                    ht[:, :, 0:H], E[:, :, 0:H], E[:, :, H:C], op=Alu.add)
                a, w = 0, H
                while w > 2:
                    w //= 2
                    nc.vector.tensor_tensor(
                        ht[:, :, a + 2 * w:a + 3 * w],
                        ht[:, :, a:a + w], ht[:, :, a + w:a + 2 * w],
                        op=Alu.add)
                    a += 2 * w
                dr = sp.tile([P, 2, cj], f32, tag="dr")
                nc.vector.tensor_tensor(dr[:, 0, :], ht[:, :, a],
                                        ht[:, :, a + 1], op=Alu.add)
                r = dr[:, 1, :]
                nc.vector.reciprocal_approx_fast(r, dr[:, 0, :])
                # [r|r] pairs so the normalize TT can read r with a
                # step-1 inner AP (keeps DVE 2x mode)
                rr2 = sp.tile([P, cj, 2], bf16, tag="rr2")
                nc.vector.tensor_copy(
                    rr2[:], r[:, :, None].to_broadcast([P, cj, 2]))

                if k + 1 < nchunk:
                    emit_exp(k + 1)

                rhs = rp.tile([P, cj, 2, C], bf16, tag="rhs")
                nc.vector.tensor_tensor(
                    rhs[:, :, 0, :].rearrange("p j (a b) -> p j a b", b=2),
                    E[:].rearrange("p j (a b) -> p j a b", b=2),
                    rr2[:, :, None, :].to_broadcast([P, cj, H, 2]),
                    op=Alu.mult)
                # square split within the chunk: ScalarE takes the first
                # s_sc tiles, DVE the rest, so both engines stay busy in
                # every chunk instead of alternating whole chunks
                s_sc = max(0, min(cj, round(sq_frac * cj)))
                if s_sc > 0:
                    nc.scalar.activation(rhs[:, 0:s_sc, 1, :],
                                         rhs[:, 0:s_sc, 0, :], Square)
                if s_sc < cj:
                    nc.vector.tensor_tensor(
                        rhs[:, s_sc:cj, 1, :], rhs[:, s_sc:cj, 0, :],
                        rhs[:, s_sc:cj, 0, :], op=Alu.mult)
                for jj in range(cj):
                    j = o + jj
                    nc.tensor.matmul(
                        psum_sq[:], M8[:, jj, :], rhs[:, jj, :, :],
                        start=(j == 0), stop=(j == j_full - 1))

            out_t = constp.tile([C, 2, C], f32)
            nc.vector.tensor_copy(out_t[:], psum_sq[:])
            nc.sync.dma_start(sq_d[:], out_t[:])

    nc.compile()
    return nc


def _get_nc():
    key = (CHUNKS, SQ_FRAC)
    if key not in _NC_CACHE:
        _NC_CACHE[key] = build_nc(CHUNKS, SQ_FRAC)
    return _NC_CACHE[key]


def kernel(column_logits, column_assignments, valid_mask):
    global LAST_RESULT
    from concourse.bass_utils import run_bass_kernel_spmd

    logits = np.asarray(column_logits, dtype=np.float32).reshape(N_TOK, C)
    seg = np.asarray(column_assignments).reshape(N_TOK).astype(np.int64)
    w = np.asarray(valid_mask).reshape(N_TOK).astype(bool)

    fp8np = ml_dtypes.float8_e4m3
    lg8 = logits.astype(fp8np)
    M8_full = np.zeros((N_TOK, C), dtype=fp8np)
    M8_full[np.arange(N_TOK)[w], seg[w]] = fp8np(1.0)   # w folded into M

    offs = [sum(CHUNKS[:k]) for k in range(len(CHUNKS))]
    in_maps = []
    for i in range(NCORES):
        sl = slice(i * TOK_PER_CORE, (i + 1) * TOK_PER_CORE)
        lgc = lg8[sl].reshape(P, J_FULL, C)      # token t = p*J_FULL + j
        m8c = M8_full[sl].reshape(P, J_FULL, C)
        lg_flat = np.empty((P, J_FULL * C), dtype=fp8np)
        m8_flat = np.empty((P, J_FULL * C), dtype=fp8np)
        for o, cj in zip(offs, CHUNKS):
            lg_flat[:, o * C:(o + cj) * C] = lgc[:, o:o + cj, :].reshape(P, cj * C)
            m8_flat[:, o * C:(o + cj) * C] = m8c[:, o:o + cj, :].reshape(P, cj * C)
        in_maps.append({"lg8": lg_flat, "m8": m8_flat})

    nc = _get_nc()
    res = run_bass_kernel_spmd(nc, in_maps, list(range(NCORES)), trace=TRACE,
                               tmpdir=TRACE_TMPDIR)
    LAST_RESULT = res

    SQ = np.zeros((C, 2, C), np.float64)
    for rm in res.results:
        SQ += np.asarray(rm["sq_out"], dtype=np.float64)
    S = SQ[:, 0, :]
    Q = SQ[:, 1, :]

    n = np.bincount(seg[w], minlength=C).astype(np.float64)
    n_safe = np.maximum(n, 1.0)
    ssd_sum = Q.sum(axis=1) - (S * S).sum(axis=1) / n_safe
    col_var = ssd_sum / (n_safe * C)
    has_multi = n > 1.0
    count = has_multi.sum()
    total = np.where(has_multi, col_var, 0.0).sum()
    loss = total / max(count, 1.0) if count > 0 else 0.0
    return np.asarray(loss, dtype=np.float32)


# revision 24
# speedup vs baseline: 1.1152x; 1.0122x over previous
"""Trainium2 Bass kernel for ColumnConsistencyLoss (segment_reduce).

Problem: B=16, T=8192, C=128.
  probs = softmax(logits, -1)           # (N, C), N = B*T = 131072
  per column-id c (segment): n_c = #valid tokens, S_c = sum w*p, Q_c = sum w*p^2
  col_var_c = (sum_j Q_cj - sum_j S_cj^2 / n_safe_c) / (n_safe_c * C)
  loss = mean over columns with n_c > 1 of col_var_c

Sharding: data-parallel over tokens - each of the 8 cores processes
N/8 = 16384 tokens and produces partial per-segment accumulators
S (C x C) and Q (C x C).  Cross-core reduction of the tiny accumulators
plus final scalar math happens on the host (n_c via host bincount).

Device kernel per core (v5 - token-major, fp8 inputs, 2x everywhere):
  Host pre-quantizes logits to fp8e4m3 (~0.5% loss error vs 2e-2 budget),
  halving DMA bytes vs f32.  Token-major layout [p, j, c] keeps the
  matmul moving operand contiguous.  The softmax-normalize broadcast
  multiply - 1x in the naive form because the per-token scalar has
  free-step 0 - runs in DVE 2x mode via a pair-replication trick:
  r is materialized as [r|r] pairs (rr2), and the TT reads it with AP
  [p, j, (64 x step0), (2 x step1)] so the packed 16-bit reads stay
  step-1 / 4B-aligned.
    ScalarE: E = exp(L8) -> bf16                     [p, j, c]
    DVE:     d = sum_c E  (6 halving adds, bf16 2x) -> fp32
             r ~= 1/d (reciprocal_approx_fast); rr2 = [r|r] bf16
             rhs[:,:,0,:] = E * rr2-trick             (TT 2x)
             rhs[:,:,1,:] = rhs0^2                    (TT 2x; some chunks
                                                      on ScalarE Square)
    PE:      psum[(c),(s,c')] += M8_j^T @ rhs[:, j, :, :]  (F=256, fp32)
  The matmul contracts the 128 partitions (tokens); w rides in the fp8
  one-hot M8: psum[c,0,:] = S_c, psum[c,1,:] = Q_c.
"""

import numpy as np
import ml_dtypes

NCORES = 8
P = 128           # partitions
C = 128           # columns / segments
H = C // 2        # 64
B, T = 16, 8192
N_TOK = B * T
TOK_PER_CORE = N_TOK // NCORES   # 16384
J_FULL = TOK_PER_CORE // P       # 128 token tiles per core
CHUNKS = (12, 24, 28, 28, 24, 12)      # token tiles per DMA/compute chunk
SQ_FRAC = 0.53    # fraction of each chunk's squares done on ScalarE

TRACE = False          # set True (e.g. from test.py) to capture NTFF profile
TRACE_TMPDIR = None    # where trace/NEFF artifacts land when TRACE is set
LAST_RESULT = None     # BassKernelResults of the last run (for profiling)

_NC_CACHE = {}


def build_nc(chunks=CHUNKS, sq_frac=SQ_FRAC):
    """Build + compile the Bass program (SPMD; same NEFF on all cores)."""
    from concourse import bacc, mybir
    import concourse.tile as tile

    f32 = mybir.dt.float32
    bf16 = mybir.dt.bfloat16
    fp8 = mybir.dt.float8e4
    Exp = mybir.ActivationFunctionType.Exp
    Square = mybir.ActivationFunctionType.Square
    Alu = mybir.AluOpType

    j_full = sum(chunks)
    tok = j_full * P

    nc = bacc.Bacc("TRN2", target_bir_lowering=False, debug=False,
                   enable_asserts=False)

    # 1-D chunk-blocked layout: chunk block = [p][j][c] contiguous, so each
    # partition slab is cj*C>=1.5KB (SDMA line rate needs >=512B) AND the
    # 128 slabs of a chunk are sequential in HBM (no 16KB-strided reads)
    lg_d = nc.dram_tensor("lg8", [tok * C], fp8, kind="ExternalInput")
    m8_d = nc.dram_tensor("m8", [tok * C], fp8, kind="ExternalInput")
    sq_d = nc.dram_tensor("sq_out", [C, 2, C], f32, kind="ExternalOutput")

    with tile.TileContext(nc) as tc:
        with (
            tc.tile_pool(name="const", bufs=1) as constp,
            tc.tile_pool(name="ld", bufs=6) as ldp,
            tc.tile_pool(name="ep", bufs=3) as ep,
            tc.tile_pool(name="rp", bufs=3) as rp,
            tc.tile_pool(name="tp", bufs=2) as tp,
            tc.tile_pool(name="sp", bufs=3) as sp,
            tc.tile_pool(name="psum", bufs=1, space="PSUM") as psump,
        ):
            psum_sq = psump.tile([C, 2, C], f32)

            nchunk = len(chunks)
            offs = [sum(chunks[:k]) for k in range(nchunk)]
            Ls = [None] * nchunk
            Ms = [None] * nchunk
            Es = [None] * nchunk

            def emit_load(k):
                cj = chunks[k]
                o = offs[k]
                base = o * C * P
                L = ldp.tile([P, cj, C], fp8, tag="L")
                nc.sync.dma_start(
                    L[:], lg_d[base:base + P * cj * C]
                    .rearrange("(p j c) -> p j c", j=cj, c=C))
                M8 = ldp.tile([P, cj, C], fp8, tag="M8")
                # gpsimd-issued DMA keeps the one-hot stream off the busy
                # Scalar queue and off the logits HWDGE ring
                nc.gpsimd.dma_start(
                    M8[:], m8_d[base:base + P * cj * C]
                    .rearrange("(p j c) -> p j c", j=cj, c=C))
                Ls[k], Ms[k] = L, M8

            def emit_exp(k):
                cj = chunks[k]
                E = ep.tile([P, cj, C], bf16, tag="E")
                nc.scalar.activation(E[:], Ls[k][:], Exp)
                Es[k] = E

            for k in range(nchunk):
                emit_load(k)
            emit_exp(0)
            for k, cj in enumerate(chunks):
                E, M8, o = Es[k], Ms[k], offs[k]
                # d = rowsum over c: 3 halving adds (bf16 2x) down to width
                # 16, then one 1x tensor_reduce -> fp32.  One scratch tile.
                ht = tp.tile([P, cj, 2 * H], bf16, tag="ht")
                nc.vector.tensor_tensor(
                    ht[:, :, 0:64], E[:, :, 0:64], E[:, :, 64:128],
                    op=Alu.add)
                nc.vector.tensor_tensor(
                    ht[:, :, 64:96], ht[:, :, 0:32], ht[:, :, 32:64],
                    op=Alu.add)
                nc.vector.tensor_tensor(
                    ht[:, :, 96:112], ht[:, :, 64:80], ht[:, :, 80:96],
                    op=Alu.add)
                dr = sp.tile([P, 2, cj], f32, tag="dr")
                nc.vector.tensor_reduce(dr[:, 0, :], ht[:, :, 96:112],
                                        axis=mybir.AxisListType.X,
                                        op=Alu.add)
                r = dr[:, 1, :]
                nc.vector.reciprocal_approx_fast(r, dr[:, 0, :])
                # [r|r] pairs so the normalize TT can read r with a
                # step-1 inner AP (keeps DVE 2x mode)
                rr2 = sp.tile([P, cj, 2], bf16, tag="rr2")
                nc.vector.tensor_copy(
                    rr2[:], r[:, :, None].to_broadcast([P, cj, 2]))

                if k + 1 < nchunk:
                    emit_exp(k + 1)

                rhs = rp.tile([P, cj, 2, C], bf16, tag="rhs")
                nc.vector.tensor_tensor(
                    rhs[:, :, 0, :].rearrange("p j (a b) -> p j a b", b=2),
                    E[:].rearrange("p j (a b) -> p j a b", b=2),
                    rr2[:, :, None, :].to_broadcast([P, cj, H, 2]),
                    op=Alu.mult)
                # square split within the chunk: DVE takes the FIRST tiles
                # (ready right after the norm, so the in-order MM FIFO can
                # flow), ScalarE the tail tiles
                s_dve = cj - max(0, min(cj, round(sq_frac * cj)))
                if s_dve > 0:
                    nc.vector.tensor_tensor(
                        rhs[:, 0:s_dve, 1, :], rhs[:, 0:s_dve, 0, :],
                        rhs[:, 0:s_dve, 0, :], op=Alu.mult)
                if s_dve < cj:
                    nc.scalar.activation(rhs[:, s_dve:cj, 1, :],
                                         rhs[:, s_dve:cj, 0, :], Square)
                for jj in range(cj):
                    j = o + jj
                    nc.tensor.matmul(
                        psum_sq[:], M8[:, jj, :], rhs[:, jj, :, :],
                        start=(j == 0), stop=(j == j_full - 1))

            out_t = constp.tile([C, 2, C], f32)
            nc.vector.tensor_copy(out_t[:], psum_sq[:])
            nc.sync.dma_start(sq_d[:], out_t[:])

    nc.compile()
    return nc


def _get_nc():
    key = (CHUNKS, SQ_FRAC)
    if key not in _NC_CACHE:
        _NC_CACHE[key] = build_nc(CHUNKS, SQ_FRAC)
    return _NC_CACHE[key]


def kernel(column_logits, column_assignments, valid_mask):
    global LAST_RESULT
    from concourse.bass_utils import run_bass_kernel_spmd

    logits = np.asarray(column_logits, dtype=np.float32).reshape(N_TOK, C)
    seg = np.asarray(column_assignments).reshape(N_TOK).astype(np.int64)
    w = np.asarray(valid_mask).reshape(N_TOK).astype(bool)

    fp8np = ml_dtypes.float8_e4m3
    lg8 = logits.astype(fp8np)
    M8_full = np.zeros((N_TOK, C), dtype=fp8np)
    M8_full[np.arange(N_TOK)[w], seg[w]] = fp8np(1.0)   # w folded into M

    offs = [sum(CHUNKS[:k]) for k in range(len(CHUNKS))]
    in_maps = []
    for i in range(NCORES):
        sl = slice(i * TOK_PER_CORE, (i + 1) * TOK_PER_CORE)
        lgc = lg8[sl].reshape(P, J_FULL, C)      # token t = p*J_FULL + j
        m8c = M8_full[sl].reshape(P, J_FULL, C)
        lg_flat = np.empty(P * J_FULL * C, dtype=fp8np)
        m8_flat = np.empty(P * J_FULL * C, dtype=fp8np)
        for o, cj in zip(offs, CHUNKS):
            base = o * C * P
            lg_flat[base:base + P * cj * C] = lgc[:, o:o + cj, :].reshape(-1)
            m8_flat[base:base + P * cj * C] = m8c[:, o:o + cj, :].reshape(-1)
        in_maps.append({"lg8": lg_flat, "m8": m8_flat})

    nc = _get_nc()
    res = run_bass_kernel_spmd(nc, in_maps, list(range(NCORES)), trace=TRACE,
                               tmpdir=TRACE_TMPDIR)
    LAST_RESULT = res

    SQ = np.zeros((C, 2, C), np.float64)
    for rm in res.results:
        SQ += np.asarray(rm["sq_out"], dtype=np.float64)
    S = SQ[:, 0, :]
    Q = SQ[:, 1, :]

    n = np.bincount(seg[w], minlength=C).astype(np.float64)
    n_safe = np.maximum(n, 1.0)
    ssd_sum = Q.sum(axis=1) - (S * S).sum(axis=1) / n_safe
    col_var = ssd_sum / (n_safe * C)
    has_multi = n > 1.0
    count = has_multi.sum()
    total = np.where(has_multi, col_var, 0.0).sum()
    loss = total / max(count, 1.0) if count > 0 else 0.0
    return np.asarray(loss, dtype=np.float32)
